# revision 39
# baseline (speedup 1.0000x reference)
"""Trainium2 Bass kernel for nn_MultiHeadAttention (B=2, T=2048, C=1024, H=16).

Sharding: 8 NeuronCores = 2 batch groups x 4 tensor-parallel cores.
Core c handles batch b = c // 4 and heads h0..h0+3, h0 = (c % 4) * 4.
Host glue: slice weights per core, sum the 4 TP partials per batch, add b_out.

Device dataflow (v1 redesign; matmuls bf16, fp32 PSUM accumulation):
  A: x [T,C] f32 -HWDGE-> SBUF f32 -DVE cast-> bf16 -xbar DMA transpose-> xT
  B: qk^T [512,T] = W_qk^T @ x^T (+bias via DVE tensor_scalar)  head-pair-major
  C: V [T,4,65] = x @ W_v (+bias, ones col)
  D: flash-style per 512-query chunk qc, per head pair:
     S^T pair = two K=64 matmuls on PE row groups 0-63/64-127 (concurrent)
     P = exp(S^T/8) one ACT op per (pair, jt); causal mask on DVE (diag only)
     O^T|colsum accumulated in PSUM over jt (ones-column trick)
     normalize: reciprocal_approx_fast on colsum row, DMA partition-broadcast,
     one DVE multiply -> ot_sb bf16
  E: y[tt,:] = ot^T @ W_out rows -> PSUM -> DVE cast-copy bf16 -> DMA out
Emission interleaves B/C(qc+1) and E(qc-1) into phase D so the PE stays busy
while ACT does exp, and the PE never idles >3.4us (HAM stays warm).
"""

import sys

sys.path.insert(0, "/opt/trn_rl_repo")

import numpy as np
import ml_dtypes

import concourse.bass as bass
import concourse.mybir as mybir
from concourse.tile import TileContext
from concourse.bass_utils import run_bass_kernel_spmd
from concourse.masks import make_identity

T = 2048
C = 1024
H = 16
D = 64
NCORE = 8
TPG = 4          # tensor-parallel group size (cores per batch)
HC = H // TPG    # heads per core
CL = HC * D      # local c dim (256)
F32 = mybir.dt.float32
BF16 = mybir.dt.bfloat16
AF = mybir.ActivationFunctionType

NT = T // 128    # 16 t-tiles
NCB = C // 128   # 8 c-tiles
NQC = T // 512   # 4 512-query chunks


def _build_program():
    nc = bass.Bass("TRN2", target_bir_lowering=False, debug=False)

    x = nc.declare_dram_parameter("x", [T, C], F32, isOutput=False)
    wqk = nc.declare_dram_parameter("wqk", [C, 2 * CL], F32, isOutput=False)
    bqk = nc.declare_dram_parameter("bqk", [2 * CL], F32, isOutput=False)
    wv = nc.declare_dram_parameter("wv", [C, CL], F32, isOutput=False)
    bv = nc.declare_dram_parameter("bv", [CL], F32, isOutput=False)
    wo = nc.declare_dram_parameter("wo", [CL, C], F32, isOutput=False)
    trimask = nc.declare_dram_parameter("trimask", [128, 128], BF16, isOutput=False)
    y = nc.declare_dram_parameter("y", [T, C], BF16, isOutput=True)

    with TileContext(nc) as tc:
        with (
            tc.tile_pool(name="singles", bufs=1) as singles,
            tc.tile_pool(name="xstage", bufs=4) as xstage,
            tc.tile_pool(name="pt", bufs=4) as ptp,
            tc.tile_pool(name="ys", bufs=4) as ysp,
            tc.tile_pool(name="wstage", bufs=4) as wstage,
            tc.tile_pool(name="rec", bufs=2) as recp,
            tc.tile_pool(name="bc", bufs=4) as bcp,
            tc.tile_pool(name="dram", bufs=2, space="DRAM") as dramp,
            tc.tile_pool(name="psum", bufs=2, space="PSUM") as pp,
        ):
            # ---- persistent SBUF tensors ----
            xT2 = singles.tile([128, NT, NCB, 128], BF16)  # x^T as [p,tt,kc,i]
            xb = singles.tile([128, NT, C], BF16)          # x cast to bf16
            qkT = singles.tile([128, 4, T], BF16)         # [q01,q23,k01,k23]
            v_sb = singles.tile([128, NT, HC, D + 1], BF16)
            ot_sb = singles.tile([128, 2, T], BF16)       # O^T norm, c_loc on part
            wqk_sb = singles.tile([128, NCB, 2 * CL], BF16)
            wv_sb = singles.tile([128, NCB, CL], BF16)
            wo_sb = singles.tile([128, 2, C], BF16)
            bqk_sb = singles.tile([128, 4], F32)
            bv_sb = singles.tile([128, CL], F32)
            mask_sb = singles.tile([128, 128], BF16)  # -240 above diagonal
            ident = singles.tile([128, 128], BF16)

            # ---- helper emitters ----
            def stage_cast(dst, src_dram, rows, cols):
                wf = wstage.tile([128, C], F32, tag="wf")
                nc.scalar.dma_start(out=wf[:, 0:cols], in_=src_dram)
                nc.vector.tensor_copy(out=dst, in_=wf[:, 0:cols])
            def emit_A_load(tt, eng=None):
                xf = xstage.tile([128, C], F32, tag="xf")
                (eng or nc.sync).dma_start(
                    out=xf, in_=x[tt * 128 : (tt + 1) * 128, :]
                )
                nc.vector.tensor_copy(out=xb[:, tt, :], in_=xf)

            def emit_A_xpose(g):
                # one xbar DMA transposes four t-tiles into xT2[:, 4g:4g+4]
                nc.sync.dma_start(
                    out=xT2[:, 4 * g : 4 * g + 4, :, :],
                    in_=xb[:, 4 * g : 4 * g + 4, :].rearrange("p a b -> p (a b)"),
                    transpose=True,
                )

            # ---- constants / weights (weights: HWDGE f32 load + DVE cast) ----
            make_identity(nc, ident)
            for tt in range(4):
                emit_A_load(tt, eng=nc.scalar)
            nc.scalar.dma_start(out=mask_sb, in_=trimask[:, :])
            for m in range(4):
                nc.scalar.dma_start(
                    out=bqk_sb[:, m : m + 1], in_=bqk[m * 128 : (m + 1) * 128, None]
                )
            nc.scalar.dma_start(out=bv_sb, in_=bv[None, :].to_broadcast((128, CL)))


            # ---- phase A: load x, cast bf16, xbar-transpose into xT.
            # Loads and xposes are emitted in separate batches so an xpose
            # waiting on its DVE cast never head-of-line-blocks later loads
            # on the SP queue. ----

            emit_A_xpose(0)
            for kc in range(NCB):
                stage_cast(
                    wqk_sb[:, kc, :], wqk[kc * 128 : (kc + 1) * 128, :], 128, 2 * CL
                )
            for tt in range(4, 8):
                emit_A_load(tt)
            for kc in range(NCB):
                stage_cast(
                    wv_sb[:, kc, :], wv[kc * 128 : (kc + 1) * 128, :], 128, CL
                )
            emit_A_xpose(1)
            for kc in range(2):
                stage_cast(wo_sb[:, kc, :], wo[kc * 128 : (kc + 1) * 128, :], 128, C)
            for tt in range(8, NT):
                emit_A_load(tt)
            for g in range(2, 4):
                emit_A_xpose(g)

            def emit_B(g):
                # qk columns for query chunk g
                for m in range(4):
                    ps = pp.tile([128, 512], F32, tag="pj", name=f"qk_{g}_{m}")
                    for kc in range(NCB):
                        nc.tensor.matmul(
                            ps[:, 0:512],
                            lhsT=wqk_sb[:, kc, m * 128 : (m + 1) * 128],
                            rhs=xT2[:, 4 * g : 4 * g + 4, kc, :],
                            start=(kc == 0),
                            stop=(kc == NCB - 1),
                        )
                    nc.vector.tensor_scalar_add(
                        out=qkT[:, m, g * 512 : (g + 1) * 512],
                        in0=ps[:, 0:512],
                        scalar1=bqk_sb[:, m : m + 1],
                    )

            def emit_C(g):
                # V rows for t-tiles of chunk g
                for tt in range(4 * g, 4 * g + 4):
                    ps = pp.tile([128, 512], F32, tag="pj", name=f"v_{tt}")
                    for kc in range(NCB):
                        nc.tensor.matmul(
                            ps[:, 0:CL],
                            lhsT=xT2[:, tt, kc, :],
                            rhs=wv_sb[:, kc, :],
                            start=(kc == 0),
                            stop=(kc == NCB - 1),
                        )
                    nc.vector.tensor_tensor(
                        out=v_sb[:, tt, :, 0:D],
                        in0=ps[:, 0:CL].rearrange("p (h d) -> p h d", h=HC),
                        in1=bv_sb.rearrange("p (h d) -> p h d", h=HC),
                        op=mybir.AluOpType.add,
                    )
                    nc.vector.memset(v_sb[:, tt, :, D : D + 1], 1.0)

            def emit_D(qc, pair):
                njt = 4 * qc + 4
                q0 = qc * 512
                ots = [
                    pp.tile([65, 512], F32, tag="ot", name=f"ot_{qc}_{pair}_{hx}")
                    for hx in range(2)
                ]
                pend = None  # (jt, pt tile, off)
                for jt in range(njt):
                    off = max(0, jt * 128 - q0)
                    stt = pp.tile(
                        [128, 2, 512], F32, tag="st", name=f"st_{qc}_{pair}_{jt}"
                    )
                    ptt = ptp.tile([128, 2, 512], BF16, tag="pt")
                    diag = jt >= 4 * qc
                    for hx in range(2):
                        pb = hx * 64
                        nc.tensor.matmul(
                            stt[:, hx, off:512],
                            lhsT=qkT[pb : pb + 64, 2 + pair, jt * 128 : (jt + 1) * 128],
                            rhs=qkT[pb : pb + 64, pair, q0 + off : q0 + 512],
                            start=True,
                            stop=not diag,
                        )
                    if diag:
                        # causal mask: add -240 above the diagonal on the PE
                        # (exp(0.125*(s-240)) ~ 0), keeping DVE off the jt path
                        for hx in range(2):
                            nc.tensor.matmul(
                                stt[:, hx, off : off + 128],
                                lhsT=mask_sb,
                                rhs=ident,
                                start=False,
                                stop=True,
                            )
                    nc.scalar.activation(
                        out=ptt[:, :, off:512],
                        in_=stt[:, :, off:512],
                        func=AF.Exp,
                        scale=0.125,
                    )
                    if pend is not None:
                        pjt, ppt, poff = pend
                        for hx in range(2):
                            nc.tensor.matmul(
                                ots[hx][:, poff:512],
                                lhsT=v_sb[:, pjt, pair * 2 + hx, :],
                                rhs=ppt[:, hx, poff:512],
                                start=(pjt == 0),
                                stop=False,
                            )
                    pend = (jt, ptt, off)
                pjt, ppt, poff = pend
                for hx in range(2):
                    nc.tensor.matmul(
                        ots[hx][:, poff:512],
                        lhsT=v_sb[:, pjt, pair * 2 + hx, :],
                        rhs=ppt[:, hx, poff:512],
                        start=(pjt == 0),
                        stop=True,
                    )
                return ots

            def emit_norm_pre(qc, pair, ots):
                # 1/Z = exp(-ln Z) on ACT (reads both heads' PSUM colsum rows
                # in one op; same table set as the softmax Exp), then one
                # DRAM-bounce broadcast for the pair.
                bcts = []
                for hx in range(2):
                    lnz = recp.tile([1, 512], F32, tag="lnz")
                    nc.scalar.activation(
                        out=lnz, in_=ots[hx][64:65, :], func=AF.Ln
                    )
                    rec = recp.tile([1, 512], F32, tag="rec")
                    nc.scalar.activation(out=rec, in_=lnz, func=AF.Exp, scale=-1.0)
                    rdr = dramp.tile([512], F32, tag="rdr")
                    nc.sync.dma_start(out=rdr[None, :], in_=rec)
                    bct = bcp.tile([64, 512], F32, tag="bc")
                    nc.sync.dma_start(
                        out=bct, in_=rdr[None, :].to_broadcast((64, 512))
                    )
                    bcts.append(bct)
                return bcts

            def emit_norm_mults(qc, pair, ots, bcts):
                # The only DVE step; emitted after filler DVE work so the
                # broadcast latency never head-of-line-blocks the DVE queue.
                for hx in range(2):
                    nc.vector.tensor_tensor(
                        out=ot_sb[
                            hx * 64 : (hx + 1) * 64,
                            pair,
                            qc * 512 : (qc + 1) * 512,
                        ],
                        in0=ots[hx][0:64, :],
                        in1=bcts[hx],
                        op=mybir.AluOpType.mult,
                    )

            def emit_E(qc, split_copies=False):
                for tt in range(4 * qc, 4 * qc + 4):
                    for nch in range(2):
                        ps = pp.tile([128, 512], F32, tag="pj", name=f"y_{tt}_{nch}")
                        for kc in range(2):
                            nc.tensor.matmul(
                                ps[:, 0:512],
                                lhsT=ot_sb[:, kc, tt * 128 : (tt + 1) * 128],
                                rhs=wo_sb[:, kc, nch * 512 : (nch + 1) * 512],
                                start=(kc == 0),
                                stop=(kc == 1),
                            )
                        ys = ysp.tile([128, 512], BF16, tag="ys")
                        if split_copies and (tt + nch) % 2 == 0:
                            nc.scalar.copy(out=ys, in_=ps[:, 0:512])
                        else:
                            nc.vector.tensor_copy(out=ys, in_=ps[:, 0:512])
                        nc.sync.dma_start(
                            out=y[
                                tt * 128 : (tt + 1) * 128,
                                nch * 512 : (nch + 1) * 512,
                            ],
                            in_=ys,
                        )

            # ---- prologue projections for chunks 0-1 (deep lookahead so
            # the small D(0)/D(1) phases never wait on B/C chains) ----
            emit_B(0)
            emit_B(1)
            emit_C(0)
            emit_C(1)

            # ---- main loop: fillers (E, B, C) placed so the PE has
            # independent work while each pair's norm chain drains ----
            for qc in range(NQC):
                ots0 = emit_D(qc, 0)
                if qc < NQC - 2:
                    emit_B(qc + 2)
                bcts0 = emit_norm_pre(qc, 0, ots0)
                if qc < NQC - 2:
                    emit_C(qc + 2)
                if qc > 0:
                    emit_E(qc - 1)
                emit_norm_mults(qc, 0, ots0, bcts0)
                ots1 = emit_D(qc, 1)
                bcts1 = emit_norm_pre(qc, 1, ots1)
                emit_norm_mults(qc, 1, ots1, bcts1)
            emit_E(NQC - 1, split_copies=True)

    _split_multi_waits(nc)
    return nc


_WAIT_CTR = [0]


def _split_multi_waits(nc, max_waits=1):
    """This container's walrus accepts only ONE sem wait per instruction.
    Hoist extra waits onto standalone EventSemaphore insts just before."""
    for f in nc.m.functions:
        for bb in f.blocks:
            insts = list(bb.instructions)
            out = []
            changed = False
            for inst in insts:
                si = inst.sync_info
                if si is not None and len(si.on_wait) > max_waits:
                    waits = list(si.on_wait)
                    keep, extra = waits[-max_waits:], waits[:-max_waits]
                    for w in extra:
                        _WAIT_CTR[0] += 1
                        out.append(
                            mybir.InstEventSemaphore(
                                name=f"xw-{_WAIT_CTR[0]}",
                                engine=inst.engine,
                                ins=[],
                                outs=[],
                                sync_info=mybir.SyncInfo(on_wait=[w], on_update=[]),
                            )
                        )
                    inst.sync_info = mybir.SyncInfo(
                        on_wait=keep, on_update=list(si.on_update)
                    )
                    changed = True
                out.append(inst)
            if changed:
                bb.instructions = out


_PROGRAM = None


def _get_program():
    global _PROGRAM
    if _PROGRAM is None:
        _PROGRAM = _build_program()
    return _PROGRAM


def _make_in_maps(x, W_attn, b_attn, W_out, b_out):
    bf16 = ml_dtypes.bfloat16
    # lhsT for the causal-mask matmul: (tri^T @ I)[j,i] = tri[i,j] = -240
    # where key j > query i (upper triangle of the lhsT itself)
    tri = (-240.0 * np.triu(np.ones((128, 128), dtype=np.float32), 1)).astype(bf16)
    in_maps = []
    for core in range(NCORE):
        b = core // TPG
        h0 = (core % TPG) * HC
        qcols = slice(h0 * D, (h0 + HC) * D)
        kcols = slice(C + h0 * D, C + (h0 + HC) * D)
        vcols = slice(2 * C + h0 * D, 2 * C + (h0 + HC) * D)
        in_maps.append(
            {
                "x": np.ascontiguousarray(x[b]),
                "wqk": np.ascontiguousarray(
                    np.concatenate([W_attn[:, qcols], W_attn[:, kcols]], axis=1)
                ),
                "bqk": np.ascontiguousarray(
                    np.concatenate([b_attn[qcols], b_attn[kcols]])
                ),
                "wv": np.ascontiguousarray(W_attn[:, vcols]),
                "bv": np.ascontiguousarray(b_attn[vcols]),
                "wo": np.ascontiguousarray(W_out[h0 * D : (h0 + HC) * D, :]),
                "trimask": tri,
            }
        )
    return in_maps


def _run(x, W_attn, b_attn, W_out, b_out, trace=False):
    nc = _get_program()
    in_maps = _make_in_maps(x, W_attn, b_attn, W_out, b_out)
    res = run_bass_kernel_spmd(nc, in_maps, list(range(NCORE)), trace=trace)
    parts = [res.results[i]["y"].astype(np.float32) for i in range(NCORE)]
    out = np.stack(
        [
            parts[0] + parts[1] + parts[2] + parts[3],
            parts[4] + parts[5] + parts[6] + parts[7],
        ]
    )
    out += b_out.astype(np.float32)
    return out, res


def kernel(x, W_attn, b_attn, W_out, b_out):
    out, _ = _run(
        np.asarray(x), np.asarray(W_attn), np.asarray(b_attn),
        np.asarray(W_out), np.asarray(b_out),
    )
    return out
